# revision 4
# baseline (speedup 1.0000x reference)
"""Grouped-channel attention (CAT FullAttention) Trainium2 kernel.

Math (per batch element b; L=S=96, R=70, E=10, P=7):
  scores[l,s,p,r] = sum_e q[l,e,p] * k[s,e,r]
  A = softmax over (s,p) of scores           (per l, r)
  out[l,e,r]      = sum_{s,p} v[s,e,p] * A[l,s,p,r]

Strategy: pure data parallel over batch (B=256 -> 32/core x 8 cores).
Per batch element on-device:
  e1  (PE) : per r, scores[s,(p,l)] = K_r^T @ Q2 in fp16 (Q pre-scaled by
             A16=128/ln2 on host so PSUM holds A16*scores), N=336 chunks in
             512-aligned PSUM slots. 14 chunks -> 5 score tiles.
  exp      : split across two engines working in parallel:
             - ACT tiles (r0-r3): true exp via activation(Exp, scale=1/A16),
               output bf16.
             - DVE tiles (r4-r6): Schraudolph bit-trick exp: one
               tensor_scalar_add(int16_out = psum + B16); the int16 result
               viewed as bf16 is 2^i*(1+f) ~= exp(x). ~3% per-weight error,
               averages out through the softmax-weighted sum.
  e2  (PE) : flipped layout: for each (r,p), matmul with the A slice
             [s=96, l=96] as stationary weights and V [s=96, e'=11] moving
             (N=11 columns only). Accumulate p in a single [96, 7*11] PSUM
             tile; V carries a ones-channel so e'=10 is the softmax
             denominator. No transposes needed: PSUM is already [l,(r,e')].
  tail(DVE): reciprocal of denominators, broadcast multiply into the
             (e,r)-swizzled output layout, contiguous DMA per batch group.
"""

import math
import sys

if "/opt/trn_rl_repo" not in sys.path:
    sys.path.insert(0, "/opt/trn_rl_repo")

import numpy as np
import ml_dtypes

import concourse.bass as bass
import concourse.bacc as bacc
import concourse.tile as tile
from concourse import mybir
from concourse.bass_utils import run_bass_kernel_spmd

B, L, R = 256, 96, 70
E, P = 10, 7
EP = E + 1  # v channels + ones channel
NCORES = 8
BPC = B // NCORES  # batches per core
G = 4  # batches per DMA group
F32 = mybir.dt.float32
F16 = mybir.dt.float16
BF16 = mybir.dt.bfloat16
I16 = mybir.dt.int16

A16 = 128.0 / math.log(2.0)  # Schraudolph scale, folded into Q on host
B16 = 16256.0 - 6.0  # 127<<7 minus calibrated bias C=6

CH = 336  # e1 chunk width: (p,l)=672 split in two
SLOT = 512  # psum slot (one bank)
# (start_chunk, n_chunks, engine): chunks jj=(r, half); ACT takes r0-r3
# (chunks 0-7), DVE Schraudolph takes r4-r6 (chunks 8-13).
TILES = [(0, 3, "act"), (3, 3, "act"), (6, 2, "act"), (8, 3, "dve"), (11, 3, "dve")]
# e2 r-groups of the previous batch emitted after e1+exp of tile t: the
# exp that frees r's chunks lands one tile earlier in the engine pipeline.
AFTER = [[0], [1, 2], [3], [4], [5, 6]]

_CACHE = {}


def _build(bpc):
    nc = bacc.Bacc("TRN2", target_bir_lowering=False, debug=False, num_devices=NCORES)
    q_d = nc.dram_tensor("q2", [bpc, E, P * L], F16, kind="ExternalInput").ap()
    k_d = nc.dram_tensor("kt", [bpc, E, P * L], F16, kind="ExternalInput").ap()
    v_d = nc.dram_tensor("vt", [bpc, L, P * EP], BF16, kind="ExternalInput").ap()
    o_d = nc.dram_tensor("out", [bpc, L, R], F32, kind="ExternalOutput").ap()

    ngroups = bpc // G

    with tile.TileContext(nc) as tc:
        with (
            tc.tile_pool(name="const", bufs=1) as cpool,
            tc.tile_pool(name="qk", bufs=2) as qkpool,
            tc.tile_pool(name="apool", bufs=3) as apool,
            tc.tile_pool(name="rd", bufs=2) as rdpool,
            tc.tile_pool(name="og", bufs=2) as ogpool,
            tc.tile_pool(name="ps_s", bufs=2, space="PSUM") as spool,
            tc.tile_pool(name="ps_e", bufs=2, space="PSUM") as e2pool,
        ):
            # touch Exp once so the ACT table set loads during the first DMA
            warm = cpool.tile([1, 2], F32)
            nc.vector.memset(warm[:, 0:1], 0.0)
            nc.scalar.activation(
                warm[:, 1:2], warm[:, 0:1], mybir.ActivationFunctionType.Exp
            )

            qg = [None] * ngroups
            kg = [None] * ngroups
            vg = [None] * ngroups
            A = [None] * bpc
            ET = [None] * bpc
            OG = [None] * ngroups

            def load_group(g):
                qt = qkpool.tile([E, G * P * L], F16, tag="qg")
                kt = qkpool.tile([E, G * P * L], F16, tag="kg")
                vt = qkpool.tile([L, G * P * EP], BF16, tag="vg")
                g0 = g * G
                if g == 0:
                    # split the first group's q/k so batch 0 is ready early
                    nc.sync.dma_start(qt[:, 0:672], q_d[g0])
                    nc.sync.dma_start(kt[:, 0:672], k_d[g0])
                    nc.sync.dma_start(
                        qt[:, 672:].rearrange("e (b f) -> e b f", b=G - 1),
                        q_d[g0 + 1 : g0 + G].rearrange("b e f -> e b f"),
                    )
                    nc.sync.dma_start(
                        kt[:, 672:].rearrange("e (b f) -> e b f", b=G - 1),
                        k_d[g0 + 1 : g0 + G].rearrange("b e f -> e b f"),
                    )
                else:
                    nc.sync.dma_start(
                        qt[:].rearrange("e (b f) -> e b f", b=G),
                        q_d[g0 : g0 + G].rearrange("b e f -> e b f"),
                    )
                    nc.sync.dma_start(
                        kt[:].rearrange("e (b f) -> e b f", b=G),
                        k_d[g0 : g0 + G].rearrange("b e f -> e b f"),
                    )
                nc.sync.dma_start(
                    vt[:].rearrange("s (b f) -> s b f", b=G),
                    v_d[g0 : g0 + G].rearrange("b s f -> s b f"),
                )
                qg[g], kg[g], vg[g] = qt, kt, vt

            def e1_tile(b, t):
                """Matmul chunks of score tile t, then exp on ACT or DVE."""
                g, i = divmod(b, G)
                qt, kt = qg[g], kg[g]
                j0, nch, eng = TILES[t]
                if t == 0:
                    at = apool.tile([L, P * P * L], BF16, tag="A")
                    A[b] = at
                at = A[b]
                st = spool.tile([L, 3 * SLOT], F32, tag="s")
                for m in range(nch):
                    jj = j0 + m
                    r, c = divmod(jj, 2)
                    nc.tensor.matmul(
                        st[:, m * SLOT : m * SLOT + CH],
                        lhsT=kt[:, i * 672 + r * L : i * 672 + (r + 1) * L],
                        rhs=qt[:, i * 672 + c * CH : i * 672 + (c + 1) * CH],
                        start=True,
                        stop=True,
                    )
                src = st[:, 0 : nch * SLOT].rearrange("s (a c) -> s a c", c=SLOT)[
                    :, :, 0:CH
                ]
                cols = slice(j0 * CH, (j0 + nch) * CH)
                if eng == "act":
                    dst = at[:, cols].rearrange("s (a c) -> s a c", c=CH)
                    nc.scalar.activation(
                        dst, src, mybir.ActivationFunctionType.Exp, scale=1.0 / A16
                    )
                else:
                    dst = at[:, cols].bitcast(I16).rearrange("s (a c) -> s a c", c=CH)
                    nc.vector.tensor_scalar_add(dst, src, B16)

            def e2_group(b, r):
                """e2 accumulation for one r: 7 matmuls, N=11 each."""
                g, i = divmod(b, G)
                vt = vg[g]
                at = A[b]
                if ET[b] is None:
                    ET[b] = e2pool.tile([L, P * EP], F32, tag="e2", name=f"et{b}")
                et = ET[b]
                for p in range(P):
                    nc.tensor.matmul(
                        et[:, r * EP : (r + 1) * EP],
                        lhsT=at[:, r * 672 + p * L : r * 672 + (p + 1) * L],
                        rhs=vt[:, i * P * EP + p * EP : i * P * EP + (p + 1) * EP],
                        start=(p == 0),
                        stop=(p == P - 1),
                    )

            def stage3(b):
                """Normalize: out[l,(e,r)] = ET[l,(r,e)] / ET[l,(r,10)]."""
                g, i = divmod(b, G)
                et = ET[b]
                t3 = et[:].rearrange("l (r e) -> l r e", e=EP)
                rd = rdpool.tile([L, P], F32, tag="rd")
                r3 = rd[:].rearrange("l (r u) -> l r u", u=1)
                nc.vector.reciprocal(r3, t3[:, :, E : E + 1])
                if OG[g] is None:
                    OG[g] = ogpool.tile([L, G * R], F32, tag="og", name=f"og{g}")
                og = OG[g]
                dst = og[:, i * R : (i + 1) * R].rearrange("l (e r) -> l r e", r=P)
                rdb = r3.copy()
                rdb.ap = rdb.ap[:-1] + [[0, E]]
                nc.vector.tensor_mul(dst, t3[:, :, 0:E], rdb)

            def flush_group(g):
                g0 = g * G
                if g == ngroups - 1:
                    # split the final flush so the tail only waits on the
                    # last batch's slice
                    nc.sync.dma_start(
                        o_d[g0 : g0 + G - 1].rearrange("b l c -> l b c"),
                        OG[g][:, 0 : (G - 1) * R].rearrange(
                            "l (b c) -> l b c", b=G - 1
                        ),
                    )
                    nc.sync.dma_start(
                        o_d[g0 + G - 1], OG[g][:, (G - 1) * R : G * R]
                    )
                else:
                    nc.sync.dma_start(
                        o_d[g0 : g0 + G].rearrange("b l c -> l b c"),
                        OG[g][:].rearrange("l (b c) -> l b c", b=G),
                    )
                OG[g] = None

            # software pipeline: e1+exp of batch b interleaved with e2 of b-1
            load_group(0)
            for b in range(bpc + 1):
                if b < bpc:
                    g, i = divmod(b, G)
                    if i == 0 and g + 1 < ngroups:
                        load_group(g + 1)
                for t in range(len(TILES)):
                    if b < bpc:
                        e1_tile(b, t)
                    if b >= 1:
                        for r in AFTER[t]:
                            e2_group(b - 1, r)
                if b >= 1:
                    stage3(b - 1)
                    if (b - 1) % G == G - 1:
                        flush_group((b - 1) // G)

    nc.compile()
    return nc


def _get_nc(bpc=BPC):
    if bpc not in _CACHE:
        _CACHE[bpc] = _build(bpc)
    return _CACHE[bpc]


def _prep(queries, keys, values):
    q = np.asarray(queries, dtype=np.float32)
    k = np.asarray(keys, dtype=np.float32)
    v = np.asarray(values, dtype=np.float32)
    b = q.shape[0]
    # Q2[b, e, p*96+l] = A16 * q[b, l, e*7+p]  (fp16)
    q2 = np.ascontiguousarray(
        (q.reshape(b, L, E, P) * A16).transpose(0, 2, 3, 1).reshape(b, E, P * L)
    ).astype(np.float16)
    # KT[b, e, r*96+s] = k[b, s, e*7+r]  (fp16)
    kt = np.ascontiguousarray(
        k.reshape(b, L, E, P).transpose(0, 2, 3, 1).reshape(b, E, P * L)
    ).astype(np.float16)
    # VT[b, s, p*11+e'] = v[b, s, e'*7+p] for e'<10, 1.0 at e'=10  (bf16)
    v4 = v.reshape(b, L, E, P).transpose(0, 1, 3, 2)  # [b, s, p, e]
    vt = np.concatenate([v4, np.ones((b, L, P, 1), np.float32)], axis=-1)
    vt = np.ascontiguousarray(vt.reshape(b, L, P * EP)).astype(ml_dtypes.bfloat16)
    return q2, kt, vt


def kernel(queries, keys, values, attn_mask=None, _trace=False):
    nc = _get_nc()
    q2, kt, vt = _prep(queries, keys, values)
    in_maps = []
    for c in range(NCORES):
        s = slice(c * BPC, (c + 1) * BPC)
        in_maps.append({"q2": q2[s], "kt": kt[s], "vt": vt[s]})
    res = None
    for attempt in range(3):
        try:
            res = run_bass_kernel_spmd(
                nc, in_maps, core_ids=list(range(NCORES)), trace=_trace
            )
            break
        except Exception:
            # shared terminal occasionally reports transient NRT device
            # errors; back off and retry
            if attempt == 2:
                raise
            import time as _time

            _time.sleep(15)
    out = np.concatenate([res.results[c]["out"] for c in range(NCORES)], axis=0)
    if _trace:
        kernel.last_exec_time_ns = res.exec_time_ns
        kernel.last_results = res
    return out.astype(np.float32)


# revision 8
# speedup vs baseline: 1.1803x; 1.1803x over previous
"""Grouped-channel attention (CAT FullAttention) Trainium2 kernel.

Math (per batch element b; L=S=96, R=70, E=10, P=7):
  scores[l,s,p,r] = sum_e q[l,e,p] * k[s,e,r]
  A = softmax over (s,p) of scores           (per l, r)
  out[l,e,r]      = sum_{s,p} v[s,e,p] * A[l,s,p,r]

Strategy: pure data parallel over batch (B=256 -> 32/core x 8 cores).
Per batch element on-device:
  e1  (PE) : per r, scores[s,(p,l)] = K_r^T @ Q2 in fp16 (Q pre-scaled by
             A16=128/ln2 on host so PSUM holds A16*scores), N=336 chunks in
             512-aligned PSUM slots. 14 chunks -> 5 score tiles.
  exp      : split across two engines working in parallel:
             - ACT tiles (r0-r3): true exp via activation(Exp, scale=1/A16),
               output bf16.
             - DVE tiles (r4-r6): Schraudolph bit-trick exp: one
               tensor_scalar_add(int16_out = psum + B16); the int16 result
               viewed as bf16 is 2^i*(1+f) ~= exp(x). ~3% per-weight error,
               averages out through the softmax-weighted sum.
  e2  (PE) : flipped layout: for each (r,p), matmul with the A slice
             [s=96, l=96] as stationary weights and V [s=96, e'=11] moving
             (N=11 columns only). Accumulate p in a single [96, 7*11] PSUM
             tile; V carries a ones-channel so e'=10 is the softmax
             denominator. No transposes needed: PSUM is already [l,(r,e')].
  tail(DVE): reciprocal of denominators, broadcast multiply into the
             (e,r)-swizzled output layout, contiguous DMA per batch group.
"""

import math
import sys

if "/opt/trn_rl_repo" not in sys.path:
    sys.path.insert(0, "/opt/trn_rl_repo")

import numpy as np
import ml_dtypes

import concourse.bass as bass
import concourse.bacc as bacc
import concourse.tile as tile
from concourse import mybir
from concourse.bass_utils import run_bass_kernel_spmd

B, L, R = 256, 96, 70
E, P = 10, 7
EP = E + 1  # v channels + ones channel
NCORES = 8
BPC = B // NCORES  # batches per core
G = 4  # batches per DMA group
F32 = mybir.dt.float32
F16 = mybir.dt.float16
BF16 = mybir.dt.bfloat16
I16 = mybir.dt.int16

A16 = 128.0 / math.log(2.0)  # Schraudolph scale, folded into Q on host
B16 = 16256.0 - 6.0  # 127<<7 minus calibrated bias C=6

# e1 emits one 2-bank PSUM window per r: matmuls of width 512+160 (each
# within a bank, union contiguous [96, 672]). One exp instruction per
# window; ACT (true exp) takes r0-r3, DVE (Schraudolph) takes r4-r6.
# bufs=3 on the window pool keeps the PE->exp->PE buffer-reuse chain
# (~2.6us/batch) shorter than the exp engines' busy time.
N_ACT_R = 4  # r windows 0..N_ACT_R-1 on ACT, rest on DVE

_CACHE = {}


def _build(bpc):
    nc = bacc.Bacc("TRN2", target_bir_lowering=False, debug=False, num_devices=NCORES)
    q_d = nc.dram_tensor("q2", [bpc, E, P * L], F16, kind="ExternalInput").ap()
    k_d = nc.dram_tensor("kt", [bpc, E, P * L], F16, kind="ExternalInput").ap()
    v_d = nc.dram_tensor("vt", [bpc, L, P * EP], BF16, kind="ExternalInput").ap()
    o_d = nc.dram_tensor("out", [bpc, L, R], F32, kind="ExternalOutput").ap()

    ngroups = bpc // G

    with tile.TileContext(nc) as tc:
        with (
            tc.tile_pool(name="const", bufs=1) as cpool,
            tc.tile_pool(name="qk", bufs=2) as qkpool,
            tc.tile_pool(name="apool", bufs=3) as apool,
            tc.tile_pool(name="rd", bufs=2) as rdpool,
            tc.tile_pool(name="og", bufs=2) as ogpool,
            tc.tile_pool(name="ps_s", bufs=3, space="PSUM") as spool,
            tc.tile_pool(name="ps_e", bufs=2, space="PSUM") as e2pool,
        ):
            # touch Exp once so the ACT table set loads during the first DMA
            warm = cpool.tile([1, 2], F32)
            nc.vector.memset(warm[:, 0:1], 0.0)
            nc.scalar.activation(
                warm[:, 1:2], warm[:, 0:1], mybir.ActivationFunctionType.Exp
            )

            qg = [None] * ngroups
            kg = [None] * ngroups
            vg = [None] * ngroups
            A = [None] * bpc
            ET = [None] * bpc
            OG = [None] * ngroups

            def load_group(g):
                qt = qkpool.tile([E, G * P * L], F16, tag="qg")
                kt = qkpool.tile([E, G * P * L], F16, tag="kg")
                vt = qkpool.tile([L, G * P * EP], BF16, tag="vg")
                g0 = g * G
                if g == 0:
                    # split the first group's q/k so batch 0 is ready early
                    nc.sync.dma_start(qt[:, 0:672], q_d[g0])
                    nc.sync.dma_start(kt[:, 0:672], k_d[g0])
                    nc.sync.dma_start(
                        qt[:, 672:].rearrange("e (b f) -> e b f", b=G - 1),
                        q_d[g0 + 1 : g0 + G].rearrange("b e f -> e b f"),
                    )
                    nc.sync.dma_start(
                        kt[:, 672:].rearrange("e (b f) -> e b f", b=G - 1),
                        k_d[g0 + 1 : g0 + G].rearrange("b e f -> e b f"),
                    )
                else:
                    nc.sync.dma_start(
                        qt[:].rearrange("e (b f) -> e b f", b=G),
                        q_d[g0 : g0 + G].rearrange("b e f -> e b f"),
                    )
                    nc.sync.dma_start(
                        kt[:].rearrange("e (b f) -> e b f", b=G),
                        k_d[g0 : g0 + G].rearrange("b e f -> e b f"),
                    )
                nc.sync.dma_start(
                    vt[:].rearrange("s (b f) -> s b f", b=G),
                    v_d[g0 : g0 + G].rearrange("b s f -> s b f"),
                )
                qg[g], kg[g], vg[g] = qt, kt, vt

            def e1_window(b, r):
                """Scores for one r (matmuls 512+160 wide) + its exp."""
                g, i = divmod(b, G)
                qt, kt = qg[g], kg[g]
                if r == 0:
                    at = apool.tile([L, P * P * L], BF16, tag="A")
                    A[b] = at
                at = A[b]
                st = spool.tile([L, 672], F32, tag="s")
                lhs = kt[:, i * 672 + r * L : i * 672 + (r + 1) * L]
                nc.tensor.matmul(
                    st[:, 0:512],
                    lhsT=lhs,
                    rhs=qt[:, i * 672 : i * 672 + 512],
                    start=True,
                    stop=True,
                )
                nc.tensor.matmul(
                    st[:, 512:672],
                    lhsT=lhs,
                    rhs=qt[:, i * 672 + 512 : i * 672 + 672],
                    start=True,
                    stop=True,
                )
                cols = slice(r * 672, (r + 1) * 672)
                if r < N_ACT_R:
                    nc.scalar.activation(
                        at[:, cols],
                        st[:],
                        mybir.ActivationFunctionType.Exp,
                        scale=1.0 / A16,
                    )
                else:
                    nc.vector.tensor_scalar_add(at[:, cols].bitcast(I16), st[:], B16)

            def e2_group(b, r):
                """e2 accumulation for one r: 7 matmuls, N=11 each."""
                g, i = divmod(b, G)
                vt = vg[g]
                at = A[b]
                if ET[b] is None:
                    ET[b] = e2pool.tile([L, P * EP], F32, tag="e2", name=f"et{b}")
                et = ET[b]
                for p in range(P):
                    nc.tensor.matmul(
                        et[:, r * EP : (r + 1) * EP],
                        lhsT=at[:, r * 672 + p * L : r * 672 + (p + 1) * L],
                        rhs=vt[:, i * P * EP + p * EP : i * P * EP + (p + 1) * EP],
                        start=(p == 0),
                        stop=(p == P - 1),
                    )

            def stage3(b):
                """Normalize: out[l,(e,r)] = ET[l,(r,e)] / ET[l,(r,10)]."""
                g, i = divmod(b, G)
                et = ET[b]
                t3 = et[:].rearrange("l (r e) -> l r e", e=EP)
                rd = rdpool.tile([L, P], F32, tag="rd")
                r3 = rd[:].rearrange("l (r u) -> l r u", u=1)
                nc.vector.reciprocal(r3, t3[:, :, E : E + 1])
                if OG[g] is None:
                    OG[g] = ogpool.tile([L, G * R], F32, tag="og", name=f"og{g}")
                og = OG[g]
                dst = og[:, i * R : (i + 1) * R].rearrange("l (e r) -> l r e", r=P)
                rdb = r3.copy()
                rdb.ap = rdb.ap[:-1] + [[0, E]]
                nc.vector.tensor_mul(dst, t3[:, :, 0:E], rdb)

            def flush_group(g):
                g0 = g * G
                if g == ngroups - 1:
                    # split the final flush so the tail only waits on the
                    # last batch's slice
                    nc.sync.dma_start(
                        o_d[g0 : g0 + G - 1].rearrange("b l c -> l b c"),
                        OG[g][:, 0 : (G - 1) * R].rearrange(
                            "l (b c) -> l b c", b=G - 1
                        ),
                    )
                    nc.sync.dma_start(
                        o_d[g0 + G - 1], OG[g][:, (G - 1) * R : G * R]
                    )
                else:
                    nc.sync.dma_start(
                        o_d[g0 : g0 + G].rearrange("b l c -> l b c"),
                        OG[g][:].rearrange("l (b c) -> l b c", b=G),
                    )
                OG[g] = None

            # software pipeline: e1+exp of batch b interleaved with e2 of b-1
            load_group(0)
            for b in range(bpc + 1):
                if b < bpc:
                    g, i = divmod(b, G)
                    if i == 0 and g + 1 < ngroups:
                        load_group(g + 1)
                for r in range(P):
                    if b < bpc:
                        e1_window(b, r)
                    if b >= 1:
                        e2_group(b - 1, r)
                if b >= 1:
                    stage3(b - 1)
                    if (b - 1) % G == G - 1:
                        flush_group((b - 1) // G)

    nc.compile()
    return nc


def _get_nc(bpc=BPC):
    if bpc not in _CACHE:
        _CACHE[bpc] = _build(bpc)
    return _CACHE[bpc]


def _prep(queries, keys, values):
    q = np.asarray(queries, dtype=np.float32)
    k = np.asarray(keys, dtype=np.float32)
    v = np.asarray(values, dtype=np.float32)
    b = q.shape[0]
    # Q2[b, e, p*96+l] = A16 * q[b, l, e*7+p]  (fp16)
    q2 = np.ascontiguousarray(
        (q.reshape(b, L, E, P) * A16).transpose(0, 2, 3, 1).reshape(b, E, P * L)
    ).astype(np.float16)
    # KT[b, e, r*96+s] = k[b, s, e*7+r]  (fp16)
    kt = np.ascontiguousarray(
        k.reshape(b, L, E, P).transpose(0, 2, 3, 1).reshape(b, E, P * L)
    ).astype(np.float16)
    # VT[b, s, p*11+e'] = v[b, s, e'*7+p] for e'<10, 1.0 at e'=10  (bf16)
    v4 = v.reshape(b, L, E, P).transpose(0, 1, 3, 2)  # [b, s, p, e]
    vt = np.concatenate([v4, np.ones((b, L, P, 1), np.float32)], axis=-1)
    vt = np.ascontiguousarray(vt.reshape(b, L, P * EP)).astype(ml_dtypes.bfloat16)
    return q2, kt, vt


def kernel(queries, keys, values, attn_mask=None, _trace=False):
    nc = _get_nc()
    q2, kt, vt = _prep(queries, keys, values)
    in_maps = []
    for c in range(NCORES):
        s = slice(c * BPC, (c + 1) * BPC)
        in_maps.append({"q2": q2[s], "kt": kt[s], "vt": vt[s]})
    res = None
    for attempt in range(3):
        try:
            res = run_bass_kernel_spmd(
                nc, in_maps, core_ids=list(range(NCORES)), trace=_trace
            )
            break
        except Exception:
            # shared terminal occasionally reports transient NRT device
            # errors; back off and retry
            if attempt == 2:
                raise
            import time as _time

            _time.sleep(15)
    out = np.concatenate([res.results[c]["out"] for c in range(NCORES)], axis=0)
    if _trace:
        kernel.last_exec_time_ns = res.exec_time_ns
        kernel.last_results = res
    return out.astype(np.float32)


# revision 9
# speedup vs baseline: 1.1816x; 1.0011x over previous
"""Grouped-channel attention (CAT FullAttention) Trainium2 kernel.

Math (per batch element b; L=S=96, R=70, E=10, P=7):
  scores[l,s,p,r] = sum_e q[l,e,p] * k[s,e,r]
  A = softmax over (s,p) of scores           (per l, r)
  out[l,e,r]      = sum_{s,p} v[s,e,p] * A[l,s,p,r]

Strategy: pure data parallel over batch (B=256 -> 32/core x 8 cores).
Per batch element on-device:
  e1  (PE) : per r, scores[s,(p,l)] = K_r^T @ Q2 in fp16 (Q pre-scaled by
             A16=128/ln2 on host so PSUM holds A16*scores), N=336 chunks in
             512-aligned PSUM slots. 14 chunks -> 5 score tiles.
  exp      : split across two engines working in parallel:
             - ACT tiles (r0-r3): true exp via activation(Exp, scale=1/A16),
               output bf16.
             - DVE tiles (r4-r6): Schraudolph bit-trick exp: one
               tensor_scalar_add(int16_out = psum + B16); the int16 result
               viewed as bf16 is 2^i*(1+f) ~= exp(x). ~3% per-weight error,
               averages out through the softmax-weighted sum.
  e2  (PE) : flipped layout: for each (r,p), matmul with the A slice
             [s=96, l=96] as stationary weights and V [s=96, e'=11] moving
             (N=11 columns only). Accumulate p in a single [96, 7*11] PSUM
             tile; V carries a ones-channel so e'=10 is the softmax
             denominator. No transposes needed: PSUM is already [l,(r,e')].
  tail(DVE): reciprocal of denominators, broadcast multiply into the
             (e,r)-swizzled output layout, contiguous DMA per batch group.
"""

import math
import sys

if "/opt/trn_rl_repo" not in sys.path:
    sys.path.insert(0, "/opt/trn_rl_repo")

import numpy as np
import ml_dtypes

import concourse.bass as bass
import concourse.bacc as bacc
import concourse.tile as tile
from concourse import mybir
from concourse.bass_utils import run_bass_kernel_spmd

B, L, R = 256, 96, 70
E, P = 10, 7
EP = E + 1  # v channels + ones channel
NCORES = 8
BPC = B // NCORES  # batches per core
G = 4  # batches per DMA group
F32 = mybir.dt.float32
F16 = mybir.dt.float16
BF16 = mybir.dt.bfloat16
I16 = mybir.dt.int16

A16 = 128.0 / math.log(2.0)  # Schraudolph scale, folded into Q on host
B16 = 16256.0 - 6.0  # 127<<7 minus calibrated bias C=6

# e1 emits one 2-bank PSUM window per r: matmuls of width 512+160 (each
# within a bank, union contiguous [96, 672]). One exp instruction per
# window; ACT (true exp) takes r0-r3, DVE (Schraudolph) takes r4-r6.
# bufs=3 on the window pool keeps the PE->exp->PE buffer-reuse chain
# (~2.6us/batch) shorter than the exp engines' busy time.
N_ACT_R = 4  # r windows 0..N_ACT_R-1 on ACT, rest on DVE

_CACHE = {}


def _build(bpc):
    nc = bacc.Bacc("TRN2", target_bir_lowering=False, debug=False, num_devices=NCORES)
    q_d = nc.dram_tensor("q2", [bpc, E, P * L], F16, kind="ExternalInput").ap()
    k_d = nc.dram_tensor("kt", [bpc, E, P * L], F16, kind="ExternalInput").ap()
    v_d = nc.dram_tensor("vt", [bpc, L, P * EP], BF16, kind="ExternalInput").ap()
    o_d = nc.dram_tensor("out", [bpc, L, R], F32, kind="ExternalOutput").ap()

    ngroups = bpc // G

    with tile.TileContext(nc) as tc:
        with (
            tc.tile_pool(name="const", bufs=1) as cpool,
            tc.tile_pool(name="qk", bufs=3) as qkpool,
            tc.tile_pool(name="apool", bufs=5) as apool,
            tc.tile_pool(name="rd", bufs=4) as rdpool,
            tc.tile_pool(name="og", bufs=3) as ogpool,
            tc.tile_pool(name="ps_s", bufs=3, space="PSUM") as spool,
            tc.tile_pool(name="ps_e", bufs=2, space="PSUM") as e2pool,
        ):
            # touch Exp once so the ACT table set loads during the first DMA
            warm = cpool.tile([1, 2], F32)
            nc.vector.memset(warm[:, 0:1], 0.0)
            nc.scalar.activation(
                warm[:, 1:2], warm[:, 0:1], mybir.ActivationFunctionType.Exp
            )

            qg = [None] * ngroups
            kg = [None] * ngroups
            vg = [None] * ngroups
            A = [None] * bpc
            ET = [None] * bpc
            OG = [None] * ngroups

            def load_group(g):
                qt = qkpool.tile([E, G * P * L], F16, tag="qg")
                kt = qkpool.tile([E, G * P * L], F16, tag="kg")
                vt = qkpool.tile([L, G * P * EP], BF16, tag="vg")
                g0 = g * G
                if g == 0:
                    # split the first group's q/k so batch 0 is ready early
                    nc.sync.dma_start(qt[:, 0:672], q_d[g0])
                    nc.sync.dma_start(kt[:, 0:672], k_d[g0])
                    nc.sync.dma_start(
                        qt[:, 672:].rearrange("e (b f) -> e b f", b=G - 1),
                        q_d[g0 + 1 : g0 + G].rearrange("b e f -> e b f"),
                    )
                    nc.sync.dma_start(
                        kt[:, 672:].rearrange("e (b f) -> e b f", b=G - 1),
                        k_d[g0 + 1 : g0 + G].rearrange("b e f -> e b f"),
                    )
                else:
                    nc.sync.dma_start(
                        qt[:].rearrange("e (b f) -> e b f", b=G),
                        q_d[g0 : g0 + G].rearrange("b e f -> e b f"),
                    )
                    nc.sync.dma_start(
                        kt[:].rearrange("e (b f) -> e b f", b=G),
                        k_d[g0 : g0 + G].rearrange("b e f -> e b f"),
                    )
                nc.sync.dma_start(
                    vt[:].rearrange("s (b f) -> s b f", b=G),
                    v_d[g0 : g0 + G].rearrange("b s f -> s b f"),
                )
                qg[g], kg[g], vg[g] = qt, kt, vt

            def e1_window(b, r):
                """Scores for one r (matmuls 512+160 wide) + its exp."""
                g, i = divmod(b, G)
                qt, kt = qg[g], kg[g]
                if r == 0:
                    at = apool.tile([L, P * P * L], BF16, tag="A")
                    A[b] = at
                at = A[b]
                st = spool.tile([L, 672], F32, tag="s")
                lhs = kt[:, i * 672 + r * L : i * 672 + (r + 1) * L]
                nc.tensor.matmul(
                    st[:, 0:512],
                    lhsT=lhs,
                    rhs=qt[:, i * 672 : i * 672 + 512],
                    start=True,
                    stop=True,
                )
                nc.tensor.matmul(
                    st[:, 512:672],
                    lhsT=lhs,
                    rhs=qt[:, i * 672 + 512 : i * 672 + 672],
                    start=True,
                    stop=True,
                )
                cols = slice(r * 672, (r + 1) * 672)
                if r < N_ACT_R:
                    nc.scalar.activation(
                        at[:, cols],
                        st[:],
                        mybir.ActivationFunctionType.Exp,
                        scale=1.0 / A16,
                    )
                else:
                    nc.vector.tensor_scalar_add(at[:, cols].bitcast(I16), st[:], B16)

            def e2_group(b, r):
                """e2 accumulation for one r: 7 matmuls, N=11 each."""
                g, i = divmod(b, G)
                vt = vg[g]
                at = A[b]
                if ET[b] is None:
                    ET[b] = e2pool.tile([L, P * EP], F32, tag="e2", name=f"et{b}")
                et = ET[b]
                for p in range(P):
                    nc.tensor.matmul(
                        et[:, r * EP : (r + 1) * EP],
                        lhsT=at[:, r * 672 + p * L : r * 672 + (p + 1) * L],
                        rhs=vt[:, i * P * EP + p * EP : i * P * EP + (p + 1) * EP],
                        start=(p == 0),
                        stop=(p == P - 1),
                    )

            def stage3(b):
                """Normalize: out[l,(e,r)] = ET[l,(r,e)] / ET[l,(r,10)]."""
                g, i = divmod(b, G)
                et = ET[b]
                t3 = et[:].rearrange("l (r e) -> l r e", e=EP)
                rd = rdpool.tile([L, P], F32, tag="rd")
                r3 = rd[:].rearrange("l (r u) -> l r u", u=1)
                nc.vector.reciprocal(r3, t3[:, :, E : E + 1])
                if OG[g] is None:
                    OG[g] = ogpool.tile([L, G * R], F32, tag="og", name=f"og{g}")
                og = OG[g]
                dst = og[:, i * R : (i + 1) * R].rearrange("l (e r) -> l r e", r=P)
                rdb = r3.copy()
                rdb.ap = rdb.ap[:-1] + [[0, E]]
                nc.vector.tensor_mul(dst, t3[:, :, 0:E], rdb)

            def flush_group(g):
                g0 = g * G
                if g == ngroups - 1:
                    # split the final flush so the tail only waits on the
                    # last batch's slice
                    nc.sync.dma_start(
                        o_d[g0 : g0 + G - 1].rearrange("b l c -> l b c"),
                        OG[g][:, 0 : (G - 1) * R].rearrange(
                            "l (b c) -> l b c", b=G - 1
                        ),
                    )
                    nc.sync.dma_start(
                        o_d[g0 + G - 1], OG[g][:, (G - 1) * R : G * R]
                    )
                else:
                    nc.sync.dma_start(
                        o_d[g0 : g0 + G].rearrange("b l c -> l b c"),
                        OG[g][:].rearrange("l (b c) -> l b c", b=G),
                    )
                OG[g] = None

            # software pipeline: e1+exp of batch b interleaved with e2 of b-1
            load_group(0)
            for b in range(bpc + 1):
                if b < bpc:
                    g, i = divmod(b, G)
                    if i == 0 and g + 1 < ngroups:
                        load_group(g + 1)
                for r in range(P):
                    if b < bpc:
                        e1_window(b, r)
                    if b >= 1:
                        e2_group(b - 1, r)
                if b >= 1:
                    stage3(b - 1)
                    if (b - 1) % G == G - 1:
                        flush_group((b - 1) // G)

    nc.compile()
    return nc


def _get_nc(bpc=BPC):
    if bpc not in _CACHE:
        _CACHE[bpc] = _build(bpc)
    return _CACHE[bpc]


def _prep(queries, keys, values):
    q = np.asarray(queries, dtype=np.float32)
    k = np.asarray(keys, dtype=np.float32)
    v = np.asarray(values, dtype=np.float32)
    b = q.shape[0]
    # Q2[b, e, p*96+l] = A16 * q[b, l, e*7+p]  (fp16)
    q2 = np.ascontiguousarray(
        (q.reshape(b, L, E, P) * A16).transpose(0, 2, 3, 1).reshape(b, E, P * L)
    ).astype(np.float16)
    # KT[b, e, r*96+s] = k[b, s, e*7+r]  (fp16)
    kt = np.ascontiguousarray(
        k.reshape(b, L, E, P).transpose(0, 2, 3, 1).reshape(b, E, P * L)
    ).astype(np.float16)
    # VT[b, s, p*11+e'] = v[b, s, e'*7+p] for e'<10, 1.0 at e'=10  (bf16)
    v4 = v.reshape(b, L, E, P).transpose(0, 1, 3, 2)  # [b, s, p, e]
    vt = np.concatenate([v4, np.ones((b, L, P, 1), np.float32)], axis=-1)
    vt = np.ascontiguousarray(vt.reshape(b, L, P * EP)).astype(ml_dtypes.bfloat16)
    return q2, kt, vt


def kernel(queries, keys, values, attn_mask=None, _trace=False):
    nc = _get_nc()
    q2, kt, vt = _prep(queries, keys, values)
    in_maps = []
    for c in range(NCORES):
        s = slice(c * BPC, (c + 1) * BPC)
        in_maps.append({"q2": q2[s], "kt": kt[s], "vt": vt[s]})
    res = None
    for attempt in range(3):
        try:
            res = run_bass_kernel_spmd(
                nc, in_maps, core_ids=list(range(NCORES)), trace=_trace
            )
            break
        except Exception:
            # shared terminal occasionally reports transient NRT device
            # errors; back off and retry
            if attempt == 2:
                raise
            import time as _time

            _time.sleep(15)
    out = np.concatenate([res.results[c]["out"] for c in range(NCORES)], axis=0)
    if _trace:
        kernel.last_exec_time_ns = res.exec_time_ns
        kernel.last_results = res
    return out.astype(np.float32)
